# revision 11
# baseline (speedup 1.0000x reference)
import sys

sys.path.insert(0, "/opt/trn_rl_repo")

import numpy as np

# Problem constants (nn_Attention_34978213658826)
B, L, DM, NH, DH = 2, 2048, 1024, 16, 64
P = 128
LT = L // P            # 16 q/k tiles
MC = DM // P           # 8 m-chunks
G = 4                  # q-tiles per group for the z matmul
NG = LT // G
HPC = 4                # heads per core
NPAIR = 2              # head pairs per core
NEG = -1.0e30
SCH = 1024             # scores psum chunk width

_CACHE = {}


def _ts(i, n):
    return slice(i * n, (i + 1) * n)


def build_bass():
    import concourse.mybir as mybir
    import concourse.tile as tile
    from concourse import bacc

    f32 = mybir.dt.float32
    f32r = mybir.dt.float32r
    bf16 = mybir.dt.bfloat16
    AX = mybir.AxisListType
    AF = mybir.ActivationFunctionType

    nc = bacc.Bacc(None, target_bir_lowering=False)
    # x^T split hi/lo in bf16 (hi + lo ~= fp32-accurate contraction, 1 cyc/row)
    xh_d = nc.dram_tensor("xh", [DM, L], bf16, kind="ExternalInput")
    xl_d = nc.dram_tensor("xl", [DM, L], bf16, kind="ExternalInput")
    wq_h = nc.dram_tensor("wqh", [NPAIR, DM + 1, P], bf16, kind="ExternalInput")
    wq_l = nc.dram_tensor("wql", [NPAIR, DM + 1, P], bf16, kind="ExternalInput")
    wk_h = nc.dram_tensor("wkh", [NPAIR, DM + 1, P], bf16, kind="ExternalInput")
    wk_l = nc.dram_tensor("wkl", [NPAIR, DM + 1, P], bf16, kind="ExternalInput")
    wv_d = nc.dram_tensor("wv", [DM + 1, HPC * DH], bf16, kind="ExternalInput")
    wo_d = nc.dram_tensor("wo", [NPAIR, P, DM], f32, kind="ExternalInput")
    msk = nc.dram_tensor("mask", [P, P], bf16, kind="ExternalInput")
    idn = nc.dram_tensor("ident", [P, P], bf16, kind="ExternalInput")
    out = nc.dram_tensor("out", [L, DM], f32, kind="ExternalOutput")

    with tile.TileContext(nc) as tc:
        with (
            tc.tile_pool(name="const", bufs=1) as const,
            tc.tile_pool(name="w", bufs=1) as wp,
            tc.tile_pool(name="qk", bufs=1) as qkp,
            tc.tile_pool(name="vz", bufs=1) as vzp,
        ):
            ident = const.tile([P, P], bf16)
            nc.sync.dma_start(ident, idn[:, :])
            mask = const.tile([P, P], bf16)
            nc.sync.dma_start(mask, msk[:, :])
            ones = const.tile([1, L], bf16)
            nc.vector.memset(ones, 1.0)

            # weights: [partition=m-row, pair, m-chunk, headcol]
            wqk = {}
            for nm, dram in (("qh", wq_h), ("ql", wq_l), ("kh", wk_h), ("kl", wk_l)):
                t = wp.tile([P, NPAIR, MC, P], bf16, name=f"w{nm}", tag=f"w{nm}")
                bb = wp.tile([1, NPAIR, P], bf16, name=f"w{nm}b", tag=f"w{nm}b")
                for _pr in range(NPAIR):
                    nc.sync.dma_start(t[:, _pr], dram[_pr, :DM, :].rearrange("(c p) h -> p c h", p=P))
                    nc.sync.dma_start(bb[:, _pr], dram[_pr, DM : DM + 1, :])
                wqk[nm] = (t, bb)
            wv_t = wp.tile([P, MC, HPC * DH], bf16)
            nc.sync.dma_start(wv_t, wv_d[:DM, :].rearrange("(c p) h -> p c h", p=P))
            wv_b = wp.tile([1, HPC * DH], bf16)
            nc.sync.dma_start(wv_b, wv_d[DM : DM + 1, :])
            wo_t = wp.tile([P, NPAIR, DM], f32r)
            for _pr in range(NPAIR):
                nc.gpsimd.dma_start(wo_t[:, _pr], wo_d[_pr, :, :])

            qT = qkp.tile([P, NPAIR, L], f32)
            kT = qkp.tile([P, NPAIR, L], f32)
            vv = vzp.tile([P, LT, HPC * DH], bf16)
            zst = [vzp.tile([P, NPAIR, G * P], f32r, name=f"zst{g}", tag=f"zst{g}") for g in range(NG)]

            # ---------------- Stage A: projections ----------------
            with (
                tc.tile_pool(name="xt", bufs=1) as xtp,
                tc.tile_pool(name="proj_ps", bufs=4, space="PSUM") as proj_ps,
                tc.tile_pool(name="v_ps", bufs=2, space="PSUM") as v_ps,
            ):
                xh = xtp.tile([P, MC, L], bf16)
                nc.sync.dma_start(xh, xh_d[:, :].rearrange("(c p) l -> p c l", p=P))
                xl = xtp.tile([P, MC, L], bf16)
                nc.sync.dma_start(xl, xl_d[:, :].rearrange("(c p) l -> p c l", p=P))

                NQ = L // 512

                def qk_proj(pr, th, tl, bh, bl, dest, scale):
                    pss = [proj_ps.tile([P, 512], f32, name=f"pp{n}", tag="pp") for n in range(NQ)]
                    for m in range(MC):
                        for n in range(NQ):
                            for lw, rx, st in (
                                (th[:, pr, m, :], xh[:, m, _ts(n, 512)], m == 0),
                                (tl[:, pr, m, :], xh[:, m, _ts(n, 512)], False),
                                (th[:, pr, m, :], xl[:, m, _ts(n, 512)], False),
                            ):
                                nc.tensor.matmul(pss[n], lhsT=lw, rhs=rx, start=st, stop=False)
                    for n in range(NQ):
                        nc.tensor.matmul(
                            pss[n], lhsT=bh[:, pr, :], rhs=ones[:, _ts(n, 512)],
                            start=False, stop=False,
                        )
                        nc.tensor.matmul(
                            pss[n], lhsT=bl[:, pr, :], rhs=ones[:, _ts(n, 512)],
                            start=False, stop=True,
                        )
                        nc.scalar.mul(dest[:, pr, _ts(n, 512)], pss[n], scale)

                def v_proj():
                    for lt in range(LT):
                        ps = v_ps.tile([P, HPC * DH], f32, name="vps", tag="vps")
                        for m in range(MC):
                            nc.tensor.matmul(
                                ps, lhsT=xh[:, m, _ts(lt, P)], rhs=wv_t[:, m, :],
                                start=(m == 0), stop=False,
                            )
                        nc.tensor.matmul(
                            ps, lhsT=ones[:, _ts(lt, P)], rhs=wv_b,
                            start=False, stop=True,
                        )
                        nc.scalar.copy(vv[:, lt, :], ps)

                qk_proj(0, wqk["qh"][0], wqk["ql"][0], wqk["qh"][1], wqk["ql"][1], qT, 0.125)
                qk_proj(0, wqk["kh"][0], wqk["kl"][0], wqk["kh"][1], wqk["kl"][1], kT, 1.0)
                v_proj()
                qk_proj(1, wqk["qh"][0], wqk["ql"][0], wqk["qh"][1], wqk["ql"][1], qT, 0.125)
                qk_proj(1, wqk["kh"][0], wqk["kl"][0], wqk["kh"][1], wqk["kl"][1], kT, 1.0)

            # ---------------- Stage B: attention per head ----------------
            with (
                tc.tile_pool(name="s_ps", bufs=3, space="PSUM") as s_ps,
                tc.tile_pool(name="z_ps", bufs=1, space="PSUM") as z_ps,
                tc.tile_pool(name="o_ps", bufs=1, space="PSUM") as o_ps,
                tc.tile_pool(name="prow", bufs=3) as prowp,
                tc.tile_pool(name="pt", bufs=2) as ptp,
                tc.tile_pool(name="stat", bufs=6) as statp,
                tc.tile_pool(name="osb", bufs=2) as osbp,
            ):
                for pr in range(NPAIR):
                    for g in range(NG):
                        ptg = [ptp.tile([P, LT, G, P], bf16, name=f"ptg{h2}", tag=f"ptg{h2}") for h2 in range(2)]
                        for s in range(G):
                            i = g * G + s
                            klen = (i + 1) * P
                            nch = (klen + SCH - 1) // SCH
                            sps2 = [[], []]
                            # interleave the two heads' chunk matmuls (K=64
                            # row-tiled pairs run concurrently on the PE)
                            for c in range(nch):
                                cw = min(SCH, klen - c * SCH)
                                dlo = klen - P - c * SCH  # diag block offset
                                has_diag = 0 <= dlo < cw
                                for h2 in range(2):
                                    sp = s_ps.tile([P, SCH], f32, name="sp", tag="s")
                                    for w0 in range(0, cw, 512):
                                        ww = min(512, cw - w0)
                                        diag_here = has_diag and w0 <= dlo < w0 + ww
                                        nc.tensor.matmul(
                                            sp[:, w0 : w0 + ww],
                                            lhsT=qT[_ts(h2, DH), pr, _ts(i, P)],
                                            rhs=kT[_ts(h2, DH), pr, c * SCH + w0 : c * SCH + w0 + ww],
                                            start=True,
                                            stop=not diag_here,
                                        )
                                        if diag_here:
                                            nc.tensor.matmul(
                                                sp[:, dlo : dlo + P],
                                                lhsT=ident,
                                                rhs=mask,
                                                start=False,
                                                stop=True,
                                            )
                                    sps2[h2].append((sp, cw))
                            for h2 in range(2):
                                sps = sps2[h2]
                                negm = statp.tile([P, 1], f32, tag="negm")
                                if nch > 1:
                                    mx = statp.tile([P, 2], f32, tag="mx")
                                    for c, (sp, cw) in enumerate(sps):
                                        nc.vector.reduce_max(
                                            mx[:, c : c + 1], sp[:, :cw], axis=AX.X
                                        )
                                    nc.vector.reduce_max(negm, mx[:, :nch], axis=AX.X, negate=True)
                                else:
                                    nc.vector.reduce_max(negm, sps[0][0][:, : sps[0][1]], axis=AX.X, negate=True)
                                prow = prowp.tile([P, L], bf16)
                                sums = statp.tile([P, 2], f32, tag="sums")
                                for c, (sp, cw) in enumerate(sps):
                                    nc.scalar.activation(
                                        prow[:, c * SCH : c * SCH + cw],
                                        sp[:, :cw],
                                        AF.Exp,
                                        bias=negm,
                                        accum_out=sums[:, c : c + 1],
                                    )
                                stot = statp.tile([P, 1], f32, tag="stot")
                                if nch > 1:
                                    nc.vector.reduce_sum(stot, sums[:, :nch], axis=AX.X)
                                else:
                                    nc.vector.tensor_copy(stot, sums[:, :1])
                                sinv = statp.tile([P, 1], f32, tag="sinv")
                                nc.vector.reciprocal(sinv, stot)
                                nc.vector.tensor_scalar_mul(
                                    prow[:, :klen], prow[:, :klen], sinv
                                )
                                nc.sync.dma_start_transpose(
                                    ptg[h2][:, : i + 1, s, :], prow[:, :klen]
                                )
                        # z matmuls for this group
                        for h2 in range(2):
                            hcol = (pr * 2 + h2) * DH
                            zps = z_ps.tile([DH, G * P], f32)
                            jmax = G * (g + 1)
                            for j in range(jmax):
                                sc = max(0, j - G * g)
                                nc.tensor.matmul(
                                    zps[:, sc * P :],
                                    lhsT=vv[:, j, hcol : hcol + DH],
                                    rhs=ptg[h2][:, j, sc:G, :],
                                    start=(j == 0),
                                    stop=(j == jmax - 1),
                                )
                            nc.scalar.copy(zst[g][_ts(h2, DH), pr, :], zps)

                # ---------------- Stage C: output projection ----------------
                for i in range(LT):
                    g, s = divmod(i, G)
                    osb = osbp.tile([P, DM], f32)
                    for mc2 in range(2):
                        ops = o_ps.tile([P, 512], f32)
                        for pr in range(NPAIR):
                            nc.tensor.matmul(
                                ops,
                                lhsT=zst[g][:, pr, _ts(s, P)],
                                rhs=wo_t[:, pr, _ts(mc2, 512)],
                                start=(pr == 0),
                                stop=(pr == 1),
                            )
                        nc.scalar.copy(osb[:, _ts(mc2, 512)], ops)
                    nc.sync.dma_start(out[_ts(i, P), :], osb)

    nc.finalize()
    return nc


def _split_bf16(a):
    import ml_dtypes

    hi = a.astype(ml_dtypes.bfloat16)
    lo = (a - hi.astype(np.float32)).astype(ml_dtypes.bfloat16)
    return hi, lo


def make_in_maps(normal_pre_resid, W_Q, W_K, W_V, W_O, b_Q, b_K, b_V, b_O):
    import ml_dtypes

    x = np.asarray(normal_pre_resid, np.float32)
    W_Q = np.asarray(W_Q, np.float32)
    W_K = np.asarray(W_K, np.float32)
    W_V = np.asarray(W_V, np.float32)
    W_O = np.asarray(W_O, np.float32)
    b_Q = np.asarray(b_Q, np.float32)
    b_K = np.asarray(b_K, np.float32)
    b_V = np.asarray(b_V, np.float32)

    mask = np.triu(np.full((P, P), NEG, np.float32), k=1).astype(ml_dtypes.bfloat16)
    ident = np.eye(P, dtype=np.float32).astype(ml_dtypes.bfloat16)
    in_maps = []
    for c in range(8):
        b, hg = divmod(c, 4)
        heads = [4 * hg + j for j in range(HPC)]
        xT = np.ascontiguousarray(x[b].T)  # [DM, L]
        xh, xl = _split_bf16(xT)

        def pack_qk(W, bias):
            prs = []
            for p_ in range(NPAIR):
                h0, h1 = heads[2 * p_], heads[2 * p_ + 1]
                wcat = np.concatenate([W[h0], W[h1]], axis=1)  # [DM, 128]
                bcat = np.concatenate([bias[h0], bias[h1]])[None, :]
                prs.append(np.concatenate([wcat, bcat], axis=0))  # [DM+1, 128]
            return _split_bf16(np.ascontiguousarray(np.stack(prs)))

        wqh, wql = pack_qk(W_Q, b_Q)
        wkh, wkl = pack_qk(W_K, b_K)
        wv_cat = np.concatenate([W_V[h] for h in heads], axis=1)
        bv_cat = np.concatenate([b_V[h] for h in heads])[None, :]
        wv_full = np.concatenate([wv_cat, bv_cat], axis=0).astype(ml_dtypes.bfloat16)
        wo_prs = np.ascontiguousarray(
            np.stack(
                [
                    np.concatenate(
                        [W_O[heads[2 * p_]], W_O[heads[2 * p_ + 1]]], axis=0
                    )
                    for p_ in range(NPAIR)
                ]
            )
        )  # [2, 128, DM]

        in_maps.append(
            {
                "xh": np.ascontiguousarray(xh),
                "xl": np.ascontiguousarray(xl),
                "wqh": wqh,
                "wql": wql,
                "wkh": wkh,
                "wkl": wkl,
                "wv": np.ascontiguousarray(wv_full),
                "wo": wo_prs,
                "mask": mask,
                "ident": ident,
            }
        )
    return in_maps


def run_device(in_maps, **kwargs):
    from concourse.bass_utils import run_bass_kernel_spmd

    if "nc" not in _CACHE:
        _CACHE["nc"] = build_bass()
    return run_bass_kernel_spmd(_CACHE["nc"], in_maps, core_ids=list(range(8)), **kwargs)


def kernel(normal_pre_resid, W_Q, W_K, W_V, W_O, b_Q, b_K, b_V, b_O, **extra):
    b_O = np.asarray(b_O, np.float32)
    in_maps = make_in_maps(
        normal_pre_resid, W_Q, W_K, W_V, W_O, b_Q, b_K, b_V, b_O
    )
    res = run_device(in_maps)
    outs = [r["out"] for r in res.results]
    full = np.zeros((B, L, DM), np.float32)
    for c in range(8):
        full[c // 4] += outs[c]
    full += b_O[None, None, :]
    return full


# revision 13
# speedup vs baseline: 1.1598x; 1.1598x over previous
import sys

sys.path.insert(0, "/opt/trn_rl_repo")

import numpy as np

# Problem constants (nn_Attention_34978213658826)
B, L, DM, NH, DH = 2, 2048, 1024, 16, 64
P = 128
LT = L // P            # 16 q/k tiles
MC = DM // P           # 8 m-chunks
G = 4                  # q-tiles per group for the z matmul
NG = LT // G
HPC = 4                # heads per core
NPAIR = 2              # head pairs per core
NEG = -1.0e30
SCH = 1024             # scores psum chunk width

_CACHE = {}


def _ts(i, n):
    return slice(i * n, (i + 1) * n)


def build_bass():
    import concourse.mybir as mybir
    import concourse.tile as tile
    from concourse import bacc

    f32 = mybir.dt.float32
    f32r = mybir.dt.float32r
    bf16 = mybir.dt.bfloat16
    AX = mybir.AxisListType
    AF = mybir.ActivationFunctionType

    nc = bacc.Bacc(None, target_bir_lowering=False)
    # x^T split hi/lo in bf16 (hi + lo ~= fp32-accurate contraction, 1 cyc/row)
    xh_d = nc.dram_tensor("xh", [DM, L], bf16, kind="ExternalInput")
    xl_d = nc.dram_tensor("xl", [DM, L], bf16, kind="ExternalInput")
    wq_h = nc.dram_tensor("wqh", [NPAIR, DM + 1, P], bf16, kind="ExternalInput")
    wq_l = nc.dram_tensor("wql", [NPAIR, DM + 1, P], bf16, kind="ExternalInput")
    wk_h = nc.dram_tensor("wkh", [NPAIR, DM + 1, P], bf16, kind="ExternalInput")
    wk_l = nc.dram_tensor("wkl", [NPAIR, DM + 1, P], bf16, kind="ExternalInput")
    wv_d = nc.dram_tensor("wv", [DM + 1, HPC * DH], bf16, kind="ExternalInput")
    wo_d = nc.dram_tensor("wo", [NPAIR, P, DM], f32, kind="ExternalInput")
    msk = nc.dram_tensor("mask", [P, P], bf16, kind="ExternalInput")
    idn = nc.dram_tensor("ident", [P, P], bf16, kind="ExternalInput")
    out = nc.dram_tensor("out", [L, DM], f32, kind="ExternalOutput")

    with tile.TileContext(nc) as tc:
        with (
            tc.tile_pool(name="const", bufs=1) as const,
            tc.tile_pool(name="w", bufs=1) as wp,
            tc.tile_pool(name="qk", bufs=1) as qkp,
            tc.tile_pool(name="vz", bufs=1) as vzp,
        ):
            ident = const.tile([P, P], bf16)
            nc.gpsimd.dma_start(ident, idn[:, :])
            mask = const.tile([P, P], bf16)
            nc.gpsimd.dma_start(mask, msk[:, :])
            ones = const.tile([1, L], bf16)
            nc.vector.memset(ones, 1.0)

            # weights: [partition=m-row, pair, m-chunk, headcol]
            wqk = {}
            for nm, dram in (("qh", wq_h), ("ql", wq_l), ("kh", wk_h), ("kl", wk_l)):
                t = wp.tile([P, NPAIR, MC, P], bf16, name=f"w{nm}", tag=f"w{nm}")
                bb = wp.tile([1, NPAIR, P], bf16, name=f"w{nm}b", tag=f"w{nm}b")
                for _pr in range(NPAIR):
                    nc.gpsimd.dma_start(t[:, _pr], dram[_pr, :DM, :].rearrange("(c p) h -> p c h", p=P))
                    nc.gpsimd.dma_start(bb[:, _pr], dram[_pr, DM : DM + 1, :])
                wqk[nm] = (t, bb)
            wv_t = wp.tile([P, MC, HPC * DH], bf16)
            nc.gpsimd.dma_start(wv_t, wv_d[:DM, :].rearrange("(c p) h -> p c h", p=P))
            wv_b = wp.tile([1, HPC * DH], bf16)
            nc.gpsimd.dma_start(wv_b, wv_d[DM : DM + 1, :])
            wo_t = wp.tile([P, NPAIR, DM], f32r)
            for _pr in range(NPAIR):
                nc.gpsimd.dma_start(wo_t[:, _pr], wo_d[_pr, :, :])

            qT = qkp.tile([P, NPAIR, L], f32)
            kT = qkp.tile([P, NPAIR, L], f32)
            vv = vzp.tile([P, LT, HPC * DH], bf16)
            zst = [vzp.tile([P, NPAIR, G * P], f32r, name=f"zst{g}", tag=f"zst{g}") for g in range(NG)]

            # ---------------- Stage A: projections ----------------
            with (
                tc.tile_pool(name="xt", bufs=1) as xtp,
                tc.tile_pool(name="proj_ps", bufs=4, space="PSUM") as proj_ps,
                tc.tile_pool(name="v_ps", bufs=2, space="PSUM") as v_ps,
            ):
                xh = xtp.tile([P, MC, L], bf16)
                nc.gpsimd.dma_start(xh, xh_d[:, :].rearrange("(c p) l -> p c l", p=P))
                xl = xtp.tile([P, MC, L], bf16)
                nc.gpsimd.dma_start(xl, xl_d[:, :].rearrange("(c p) l -> p c l", p=P))

                NQ = L // 512

                def qk_proj(pr, th, tl, bh, bl, dest, scale):
                    pss = [proj_ps.tile([P, 512], f32, name=f"pp{n}", tag="pp") for n in range(NQ)]
                    for m in range(MC):
                        for vi, (lw, rx) in enumerate((
                            (th[:, pr, m, :], xh),
                            (tl[:, pr, m, :], xh),
                            (th[:, pr, m, :], xl),
                        )):
                            for n in range(NQ):
                                nc.tensor.matmul(
                                    pss[n], lhsT=lw, rhs=rx[:, m, _ts(n, 512)],
                                    start=(m == 0 and vi == 0), stop=False,
                                )
                    for n in range(NQ):
                        nc.tensor.matmul(
                            pss[n], lhsT=bh[:, pr, :], rhs=ones[:, _ts(n, 512)],
                            start=False, stop=False,
                        )
                        nc.tensor.matmul(
                            pss[n], lhsT=bl[:, pr, :], rhs=ones[:, _ts(n, 512)],
                            start=False, stop=True,
                        )
                        nc.scalar.mul(dest[:, pr, _ts(n, 512)], pss[n], scale)

                def v_proj():
                    for lt in range(LT):
                        ps = v_ps.tile([P, HPC * DH], f32, name="vps", tag="vps")
                        for m in range(MC):
                            nc.tensor.matmul(
                                ps, lhsT=xh[:, m, _ts(lt, P)], rhs=wv_t[:, m, :],
                                start=(m == 0), stop=False,
                            )
                        nc.tensor.matmul(
                            ps, lhsT=ones[:, _ts(lt, P)], rhs=wv_b,
                            start=False, stop=True,
                        )
                        nc.scalar.copy(vv[:, lt, :], ps)

                qk_proj(0, wqk["qh"][0], wqk["ql"][0], wqk["qh"][1], wqk["ql"][1], qT, 0.125)
                qk_proj(0, wqk["kh"][0], wqk["kl"][0], wqk["kh"][1], wqk["kl"][1], kT, 1.0)
                v_proj()
                qk_proj(1, wqk["qh"][0], wqk["ql"][0], wqk["qh"][1], wqk["ql"][1], qT, 0.125)
                qk_proj(1, wqk["kh"][0], wqk["kl"][0], wqk["kh"][1], wqk["kl"][1], kT, 1.0)

            # ---------------- Stage B: attention per head ----------------
            with (
                tc.tile_pool(name="s_ps", bufs=3, space="PSUM") as s_ps,
                tc.tile_pool(name="z_ps", bufs=1, space="PSUM") as z_ps,
                tc.tile_pool(name="o_ps", bufs=1, space="PSUM") as o_ps,
                tc.tile_pool(name="prow", bufs=4) as prowp,
                tc.tile_pool(name="pt", bufs=2) as ptp,
                tc.tile_pool(name="stat", bufs=6) as statp,
                tc.tile_pool(name="osb", bufs=2) as osbp,
            ):
                ptgs = {}

                def emit_S(pr, g):
                    ptg = [ptp.tile([P, LT, G, P], bf16, name=f"ptg{h2}", tag=f"ptg{h2}") for h2 in range(2)]
                    ptgs[(pr, g)] = ptg
                    for s in range(G):
                        i = g * G + s
                        klen = (i + 1) * P
                        nch = (klen + SCH - 1) // SCH
                        sps2 = [[], []]
                        # interleave the two heads' chunk matmuls (K=64
                        # row-tiled pairs run concurrently on the PE)
                        for c in range(nch):
                            cw = min(SCH, klen - c * SCH)
                            dlo = klen - P - c * SCH  # diag block offset
                            has_diag = 0 <= dlo < cw
                            for h2 in range(2):
                                sp = s_ps.tile([P, SCH], f32, name="sp", tag="s")
                                for w0 in range(0, cw, 512):
                                    ww = min(512, cw - w0)
                                    diag_here = has_diag and w0 <= dlo < w0 + ww
                                    nc.tensor.matmul(
                                        sp[:, w0 : w0 + ww],
                                        lhsT=qT[_ts(h2, DH), pr, _ts(i, P)],
                                        rhs=kT[_ts(h2, DH), pr, c * SCH + w0 : c * SCH + w0 + ww],
                                        start=True,
                                        stop=not diag_here,
                                    )
                                    if diag_here:
                                        nc.tensor.matmul(
                                            sp[:, dlo : dlo + P],
                                            lhsT=ident,
                                            rhs=mask,
                                            start=False,
                                            stop=True,
                                        )
                                sps2[h2].append((sp, cw))
                        for h2 in range(2):
                            sps = sps2[h2]
                            negm = statp.tile([P, 1], f32, tag="negm")
                            if nch > 1:
                                mx = statp.tile([P, 2], f32, tag="mx")
                                for c, (sp, cw) in enumerate(sps):
                                    nc.vector.reduce_max(
                                        mx[:, c : c + 1], sp[:, :cw], axis=AX.X
                                    )
                                nc.vector.reduce_max(negm, mx[:, :nch], axis=AX.X, negate=True)
                            else:
                                nc.vector.reduce_max(negm, sps[0][0][:, : sps[0][1]], axis=AX.X, negate=True)
                            prow = prowp.tile([P, L], bf16)
                            sums = statp.tile([P, 2], f32, tag="sums")
                            for c, (sp, cw) in enumerate(sps):
                                nc.scalar.activation(
                                    prow[:, c * SCH : c * SCH + cw],
                                    sp[:, :cw],
                                    AF.Exp,
                                    bias=negm,
                                    accum_out=sums[:, c : c + 1],
                                )
                            stot = statp.tile([P, 1], f32, tag="stot")
                            if nch > 1:
                                nc.vector.reduce_sum(stot, sums[:, :nch], axis=AX.X)
                            else:
                                nc.vector.tensor_copy(stot, sums[:, :1])
                            sinv = statp.tile([P, 1], f32, tag="sinv")
                            nc.vector.reciprocal(sinv, stot)
                            nc.vector.tensor_scalar_mul(
                                prow[:, :klen], prow[:, :klen], sinv
                            )
                            nc.sync.dma_start_transpose(
                                ptg[h2][:, : i + 1, s, :], prow[:, :klen]
                            )

                def emit_Z(pr, g):
                    ptg = ptgs.pop((pr, g))
                    for h2 in range(2):
                        hcol = (pr * 2 + h2) * DH
                        zps = z_ps.tile([DH, G * P], f32)
                        jmax = G * (g + 1)
                        for j in range(jmax):
                            sc = max(0, j - G * g)
                            nc.tensor.matmul(
                                zps[:, sc * P :],
                                lhsT=vv[:, j, hcol : hcol + DH],
                                rhs=ptg[h2][:, j, sc:G, :],
                                start=(j == 0),
                                stop=(j == jmax - 1),
                            )
                        nc.scalar.copy(zst[g][_ts(h2, DH), pr, :], zps)

                def emit_O(g):
                    for s in range(G):
                        i = g * G + s
                        osb = osbp.tile([P, DM], f32)
                        for mc2 in range(2):
                            ops = o_ps.tile([P, 512], f32)
                            for pr in range(NPAIR):
                                nc.tensor.matmul(
                                    ops,
                                    lhsT=zst[g][:, pr, _ts(s, P)],
                                    rhs=wo_t[:, pr, _ts(mc2, 512)],
                                    start=(pr == 0),
                                    stop=(pr == 1),
                                )
                            nc.scalar.copy(osb[:, _ts(mc2, 512)], ops)
                        nc.gpsimd.dma_start(out[_ts(i, P), :], osb)

                # software pipeline: z of group (pr,g) is emitted while the
                # NEXT group's scores run, so the PE never stalls on the
                # softmax/transpose chain; out-proj follows the pr=1 z.
                prev = None
                for pr in range(NPAIR):
                    for g in range(NG):
                        emit_S(pr, g)
                        if prev is not None:
                            emit_Z(*prev)
                            if prev[0] == 1:
                                emit_O(prev[1])
                        prev = (pr, g)
                emit_Z(*prev)
                emit_O(prev[1])

    nc.finalize()
    return nc


def _split_bf16(a):
    import ml_dtypes

    hi = a.astype(ml_dtypes.bfloat16)
    lo = (a - hi.astype(np.float32)).astype(ml_dtypes.bfloat16)
    return hi, lo


def make_in_maps(normal_pre_resid, W_Q, W_K, W_V, W_O, b_Q, b_K, b_V, b_O):
    import ml_dtypes

    x = np.asarray(normal_pre_resid, np.float32)
    W_Q = np.asarray(W_Q, np.float32)
    W_K = np.asarray(W_K, np.float32)
    W_V = np.asarray(W_V, np.float32)
    W_O = np.asarray(W_O, np.float32)
    b_Q = np.asarray(b_Q, np.float32)
    b_K = np.asarray(b_K, np.float32)
    b_V = np.asarray(b_V, np.float32)

    mask = np.triu(np.full((P, P), NEG, np.float32), k=1).astype(ml_dtypes.bfloat16)
    ident = np.eye(P, dtype=np.float32).astype(ml_dtypes.bfloat16)
    in_maps = []
    for c in range(8):
        b, hg = divmod(c, 4)
        heads = [4 * hg + j for j in range(HPC)]
        xT = np.ascontiguousarray(x[b].T)  # [DM, L]
        xh, xl = _split_bf16(xT)

        def pack_qk(W, bias):
            prs = []
            for p_ in range(NPAIR):
                h0, h1 = heads[2 * p_], heads[2 * p_ + 1]
                wcat = np.concatenate([W[h0], W[h1]], axis=1)  # [DM, 128]
                bcat = np.concatenate([bias[h0], bias[h1]])[None, :]
                prs.append(np.concatenate([wcat, bcat], axis=0))  # [DM+1, 128]
            return _split_bf16(np.ascontiguousarray(np.stack(prs)))

        wqh, wql = pack_qk(W_Q, b_Q)
        wkh, wkl = pack_qk(W_K, b_K)
        wv_cat = np.concatenate([W_V[h] for h in heads], axis=1)
        bv_cat = np.concatenate([b_V[h] for h in heads])[None, :]
        wv_full = np.concatenate([wv_cat, bv_cat], axis=0).astype(ml_dtypes.bfloat16)
        wo_prs = np.ascontiguousarray(
            np.stack(
                [
                    np.concatenate(
                        [W_O[heads[2 * p_]], W_O[heads[2 * p_ + 1]]], axis=0
                    )
                    for p_ in range(NPAIR)
                ]
            )
        )  # [2, 128, DM]

        in_maps.append(
            {
                "xh": np.ascontiguousarray(xh),
                "xl": np.ascontiguousarray(xl),
                "wqh": wqh,
                "wql": wql,
                "wkh": wkh,
                "wkl": wkl,
                "wv": np.ascontiguousarray(wv_full),
                "wo": wo_prs,
                "mask": mask,
                "ident": ident,
            }
        )
    return in_maps


def run_device(in_maps, **kwargs):
    from concourse.bass_utils import run_bass_kernel_spmd

    if "nc" not in _CACHE:
        _CACHE["nc"] = build_bass()
    return run_bass_kernel_spmd(_CACHE["nc"], in_maps, core_ids=list(range(8)), **kwargs)


def kernel(normal_pre_resid, W_Q, W_K, W_V, W_O, b_Q, b_K, b_V, b_O, **extra):
    b_O = np.asarray(b_O, np.float32)
    in_maps = make_in_maps(
        normal_pre_resid, W_Q, W_K, W_V, W_O, b_Q, b_K, b_V, b_O
    )
    res = run_device(in_maps)
    outs = [r["out"] for r in res.results]
    full = np.zeros((B, L, DM), np.float32)
    for c in range(8):
        full[c // 4] += outs[c]
    full += b_O[None, None, :]
    return full
